# revision 1
# baseline (speedup 1.0000x reference)
"""Soft-DTW ranking loss kernel for Trainium2 (8 NeuronCores, SPMD data parallel).

Math: loss = mean((diff - labels)^2) where
  diff_b = sdtw(OTH_b,X_b) - sdtw(TGT_b,X_b) - 0.5*sdtw(OTH_b,OTH_b) + 0.5*sdtw(TGT_b,TGT_b)
(the sdtw(X,X) terms of the normalized soft-DTW cancel exactly).

Soft-DTW (gamma=1) is computed in the probability domain:
  E[i,j] = K[i,j] * (E[i-1,j] + E[i-1,j-1] + E[i,j-1]),  K = exp(<xn_i,yn_j> - 1)
which maps one DP row onto a single DVE tensor_tensor_scan:
  state = (s[t] + state) * K[t],   s = E_prev + shift1(E_prev)
with periodic per-instance rescaling (log-scale accumulated in C) to stay in
fp32 range. Each core handles 8 batch items x 4 DTW instances = 32 independent
DPs vectorized across SBUF partitions.
"""

import os
import sys

import numpy as np

for _p in ("/root/.axon_site", "/root/.axon_site/_ro/trn_rl_repo",
           "/root/.axon_site/_ro/pypackages", "/opt/trn_rl_repo", "/opt/pypackages"):
    if os.path.isdir(_p) and _p not in sys.path:
        sys.path.append(_p)

import concourse.bass as bass
import concourse.tile as tile
from concourse.tile import add_dep_helper
from concourse import bacc, mybir
from concourse.bass_utils import run_bass_kernel_spmd
from concourse.masks import make_identity

F32 = mybir.dt.float32
F32R = mybir.dt.float32r
AX = mybir.AxisListType
OP = mybir.AluOpType
AF = mybir.ActivationFunctionType

B, T, D = 64, 512, 64
NCORES = 8
BPC = B // NCORES          # batch items per core
NTYPE = 4                  # (OTH,X), (TGT,X), (OTH,OTH), (TGT,TGT)
RESC = 32                  # rescale cadence (rows)


def _emit(tc: tile.TileContext, ins: dict, outs: dict, kbuf: bass.AP,
          t_len: int, bpc: int, resc: int):
    nc = tc.nc
    ni = NTYPE * bpc
    nrowt = t_len // 128

    with (
        tc.tile_pool(name="const", bufs=1) as p_const,
        tc.tile_pool(name="ain", bufs=2) as p_in,
        tc.tile_pool(name="astat", bufs=2) as p_astat,
        tc.tile_pool(name="asn", bufs=2) as p_asn,
        tc.tile_pool(name="apsT", bufs=2, space="PSUM") as p_psT,
        tc.tile_pool(name="ant", bufs=2) as p_nt,
        tc.tile_pool(name="aG", bufs=2, space="PSUM") as p_G,
        tc.tile_pool(name="aK", bufs=3) as p_K,
        tc.tile_pool(name="bE", bufs=1) as p_E,
        tc.tile_pool(name="bS", bufs=2) as p_s,
        tc.tile_pool(name="bK", bufs=4) as p_k,
        tc.tile_pool(name="bstat", bufs=2) as p_stat,
        tc.tile_pool(name="bacc", bufs=1) as p_acc,
    ):
        ident = p_const.tile([128, 128], F32, tag="ident")
        make_identity(nc, ident[:])
        bias_m1 = p_const.tile([128, 1], F32, tag="biasm1")
        nc.gpsimd.memset(bias_m1[:], -1.0)

        # ---------------- Phase A: K = exp(<xn,yn> - 1) for all pairs -------
        for b in range(bpc):
            nT = {}
            for sname in ("OTH", "TGT", "X"):
                src = ins[sname]
                xin = p_in.tile([128, nrowt * D], F32, tag=f"in_{sname}")
                nc.sync.dma_start(
                    xin[:].rearrange("p (t d) -> p t d", d=D),
                    src[b].rearrange("(t p) d -> p t d", p=128),
                )
                sq = p_astat.tile([128, nrowt * D], F32, tag=f"sq_{sname}")
                ss = p_astat.tile([128, nrowt], F32, tag=f"ss_{sname}")
                for t in range(nrowt):
                    nc.scalar.activation(
                        sq[:, t * D:(t + 1) * D], xin[:, t * D:(t + 1) * D],
                        AF.Square, accum_out=ss[:, t:t + 1],
                    )
                nrm = p_astat.tile([128, nrowt], F32, tag=f"nrm_{sname}")
                nc.scalar.activation(nrm[:], ss[:], AF.Sqrt)
                rnm = p_astat.tile([128, nrowt], F32, tag=f"rnm_{sname}")
                nc.vector.reciprocal(rnm[:], nrm[:])
                sn = p_asn.tile([128, nrowt * D], F32, tag=f"sn_{sname}")
                for t in range(nrowt):
                    nc.vector.tensor_scalar_mul(
                        sn[:, t * D:(t + 1) * D], xin[:, t * D:(t + 1) * D],
                        rnm[:, t:t + 1],
                    )
                snT = p_nt.tile([D, t_len], F32R, tag=f"nt_{sname}")
                for t in range(nrowt):
                    tp = p_psT.tile([D, 128], F32, tag="psT")
                    nc.tensor.transpose(tp[:], sn[:, t * D:(t + 1) * D], ident[:])
                    nc.scalar.copy(snT[:, t * 128:(t + 1) * 128], tp[:])
                nT[sname] = snT

            pairs = [("OTH", "X"), ("TGT", "X"), ("OTH", "OTH"), ("TGT", "TGT")]
            for ptype, (an, cn) in enumerate(pairs):
                inst = ptype * bpc + b
                aT, cT = nT[an], nT[cn]
                for rt in range(nrowt):
                    g = p_G.tile([128, t_len], F32, tag="G")
                    nc.tensor.matmul(
                        g[:],
                        aT[:, rt * 128:(rt + 1) * 128],
                        cT[:],
                        start=True, stop=True,
                    )
                    kt = p_K.tile([128, t_len], F32, tag="K")
                    nc.scalar.activation(kt[:], g[:], AF.Exp, bias=bias_m1[:])
                    nc.sync.dma_start(kbuf[inst, rt * 128:(rt + 1) * 128, :], kt[:])

        # ---------------- Phase B: row-scan DP over all instances -----------
        Ea = p_E.tile([ni, t_len + 1], F32, tag="Ea")
        Eb = p_E.tile([ni, t_len + 1], F32, tag="Eb")
        cacc = p_acc.tile([ni, 1], F32, tag="C")
        nc.gpsimd.memset(Ea[:], 0.0)
        nc.gpsimd.memset(Eb[:], 0.0)
        nc.gpsimd.memset(cacc[:], 0.0)
        nc.gpsimd.memset(Ea[:, 0:1], 1.0)  # E[-1][-1] = exp(-0)

        cur, nxt = Ea, Eb
        for r in range(t_len):
            kt = p_k.tile([ni, t_len], F32, tag="krow")
            nc.sync.dma_start(kt[:], kbuf[:, r, :])
            s = p_s.tile([ni, t_len], F32, tag="s")
            nc.vector.tensor_add(s[:], cur[:, 1:t_len + 1], cur[:, 0:t_len])
            nc.vector.tensor_tensor_scan(
                nxt[:, 1:t_len + 1], s[:], kt[:], 0.0, OP.add, OP.mult,
            )
            if r == 0:
                # E[0][-1] = 0: clear the one-time E[-1][-1] = 1 boundary
                nc.vector.memset(Ea[:, 0:1], 0.0)
            if (r + 1) % resc == 0 and r != t_len - 1:
                mx = p_stat.tile([ni, 1], F32, tag="mx")
                nc.vector.tensor_reduce(mx[:], nxt[:, 1:t_len + 1], AX.X, OP.max)
                rec = p_stat.tile([ni, 1], F32, tag="rec")
                nc.vector.reciprocal(rec[:], mx[:])
                nc.vector.tensor_scalar_mul(nxt[:, 1:t_len + 1],
                                            nxt[:, 1:t_len + 1], rec[:])
                lg = p_stat.tile([ni, 1], F32, tag="lg")
                nc.scalar.activation(lg[:], mx[:], AF.Ln)
                nc.vector.tensor_add(cacc[:], cacc[:], lg[:])
            cur, nxt = nxt, cur

        nc.sync.dma_start(outs["EOUT"].rearrange("(a b) -> a b", b=1),
                          cur[:, t_len:t_len + 1])
        nc.sync.dma_start(outs["COUT"].rearrange("(a b) -> a b", b=1), cacc[:])


def _emit_wave(tc: tile.TileContext, ins: dict, outs: dict, kbuf: bass.AP,
               t_len: int, bpc: int, resc: int):
    """Wavefront DP: CH=t_len/128 column chunks on partition groups.

    Partition p = g*ni + inst handles column chunk g of instance inst.
    Wavefront step w: group g processes row r = w - g (K rows padded with 3
    zero rows on each side so inactive groups compute zeros). Cross-chunk
    carries (scan initial / shifted-row boundary) move between partition
    groups via a constant shift matmul on the (otherwise idle) PE.
    """
    nc = tc.nc
    ni = NTYPE * bpc
    ch = t_len // 128
    npart = ch * ni
    nrowt = ch
    nsteps = t_len + ch - 1

    with (
        tc.tile_pool(name="const", bufs=1) as p_const,
        tc.tile_pool(name="ain", bufs=2) as p_in,
        tc.tile_pool(name="astat", bufs=2) as p_astat,
        tc.tile_pool(name="asn", bufs=2) as p_asn,
        tc.tile_pool(name="apsT", bufs=2, space="PSUM") as p_psT,
        tc.tile_pool(name="ant", bufs=2) as p_nt,
        tc.tile_pool(name="aG", bufs=2, space="PSUM") as p_G,
        tc.tile_pool(name="aK", bufs=3) as p_K,
        tc.tile_pool(name="bE", bufs=1) as p_E,
        tc.tile_pool(name="bS", bufs=2) as p_s,
        tc.tile_pool(name="bK", bufs=8) as p_k,
        tc.tile_pool(name="bC", bufs=3, space="PSUM") as p_carry,
        tc.tile_pool(name="bB", bufs=1, space="PSUM") as p_bc,
        tc.tile_pool(name="bstat", bufs=2) as p_stat,
        tc.tile_pool(name="bacc", bufs=1) as p_acc,
    ):
        ident = p_const.tile([128, 128], F32, tag="ident")
        make_identity(nc, ident[:])
        bias_m1 = p_const.tile([128, 1], F32, tag="biasm1")
        nc.gpsimd.memset(bias_m1[:], -1.0)
        # shiftM[k, p] = 1 iff k == p - ni  (moves group g-1 -> g)
        shiftM = p_const.tile([npart, npart], F32, tag="shiftM")
        nc.gpsimd.memset(shiftM[:], 0.0)
        nc.gpsimd.affine_select(
            out=shiftM[:], in_=shiftM[:], compare_op=OP.not_equal, fill=1.0,
            base=ni, pattern=[[-1, npart]], channel_multiplier=1,
        )
        # bcastM[k, (g, j)] = 1 iff k == j  (broadcast group-0 col to all groups)
        bcastM = p_const.tile([ni, npart], F32, tag="bcastM")
        nc.gpsimd.memset(bcastM[:], 0.0)
        nc.gpsimd.affine_select(
            out=bcastM[:].rearrange("k (g j) -> k g j", j=ni),
            in_=bcastM[:].rearrange("k (g j) -> k g j", j=ni),
            compare_op=OP.not_equal, fill=1.0,
            base=0, pattern=[[0, ch], [-1, ni]], channel_multiplier=1,
        )

        # zero the 3+3 pad rows of kbuf (layout [ni, t_len+6, t_len])
        zpad = p_const.tile([ni, 3 * t_len], F32, tag="zpad")
        nc.gpsimd.memset(zpad[:], 0.0)
        nc.sync.dma_start(
            kbuf[:, 0:3, :].rearrange("i r c -> i (r c)"), zpad[:])
        nc.sync.dma_start(
            kbuf[:, t_len + 3:t_len + 6, :].rearrange("i r c -> i (r c)"), zpad[:])

        # ---------------- Phase A (same as v1, +3 row offset into kbuf) -----
        for b in range(bpc):
            nT = {}
            for sname in ("OTH", "TGT", "X"):
                src = ins[sname]
                xin = p_in.tile([128, nrowt * D], F32, tag=f"in_{sname}")
                nc.sync.dma_start(
                    xin[:].rearrange("p (t d) -> p t d", d=D),
                    src[b].rearrange("(t p) d -> p t d", p=128),
                )
                sq = p_astat.tile([128, nrowt * D], F32, tag=f"sq_{sname}")
                ss = p_astat.tile([128, nrowt], F32, tag=f"ss_{sname}")
                for t in range(nrowt):
                    nc.scalar.activation(
                        sq[:, t * D:(t + 1) * D], xin[:, t * D:(t + 1) * D],
                        AF.Square, accum_out=ss[:, t:t + 1],
                    )
                nrm = p_astat.tile([128, nrowt], F32, tag=f"nrm_{sname}")
                nc.scalar.activation(nrm[:], ss[:], AF.Sqrt)
                rnm = p_astat.tile([128, nrowt], F32, tag=f"rnm_{sname}")
                nc.vector.reciprocal(rnm[:], nrm[:])
                sn = p_asn.tile([128, nrowt * D], F32, tag=f"sn_{sname}")
                for t in range(nrowt):
                    nc.vector.tensor_scalar_mul(
                        sn[:, t * D:(t + 1) * D], xin[:, t * D:(t + 1) * D],
                        rnm[:, t:t + 1],
                    )
                snT = p_nt.tile([D, t_len], F32R, tag=f"nt_{sname}")
                for t in range(nrowt):
                    tp = p_psT.tile([D, 128], F32, tag="psT")
                    nc.tensor.transpose(tp[:], sn[:, t * D:(t + 1) * D], ident[:])
                    nc.scalar.copy(snT[:, t * 128:(t + 1) * 128], tp[:])
                nT[sname] = snT

            pairs = [("OTH", "X"), ("TGT", "X"), ("OTH", "OTH"), ("TGT", "TGT")]
            for ptype, (an, cn) in enumerate(pairs):
                inst = ptype * bpc + b
                aT, cT = nT[an], nT[cn]
                for rt in range(nrowt):
                    g = p_G.tile([128, t_len], F32, tag="G")
                    nc.tensor.matmul(
                        g[:], aT[:, rt * 128:(rt + 1) * 128], cT[:],
                        start=True, stop=True,
                    )
                    kt = p_K.tile([128, t_len], F32, tag="K")
                    nc.scalar.activation(kt[:], g[:], AF.Exp, bias=bias_m1[:])
                    nc.sync.dma_start(
                        kbuf[inst, 3 + rt * 128:3 + (rt + 1) * 128, :], kt[:])

        # ---------------- Phase B: wavefront row-scan -----------------------
        Ea = p_E.tile([npart, 129], F32, tag="Ea")
        Eb = p_E.tile([npart, 129], F32, tag="Eb")
        Etiles = [Ea, Eb]
        cacc = p_acc.tile([npart, 1], F32, tag="C")
        nc.gpsimd.memset(Ea[:], 0.0)
        nc.gpsimd.memset(Eb[:], 0.0)
        nc.gpsimd.memset(cacc[:], 0.0)
        nc.gpsimd.memset(Ea[0:ni, 0:1], 1.0)  # E[-1][-1] = 1 for group 0
        car_prev = p_carry.tile([npart, 1], F32, tag="car")
        car_prev_mm = nc.vector.memset(car_prev[:], 0.0)

        for w in range(nsteps):
            prev = Etiles[w % 2]
            newt = Etiles[(w + 1) % 2]
            kt = p_k.tile([npart, 128], F32, tag="krow")
            for g in range(ch):
                nc.sync.dma_start(
                    kt[g * ni:(g + 1) * ni, :],
                    kbuf[:, w - g + 3, g * 128:(g + 1) * 128],
                )
            s = p_s.tile([npart, 128], F32, tag="s")
            nc.vector.tensor_add(s[:], prev[:, 1:129], prev[:, 0:128])
            scan_i = nc.vector.tensor_tensor_scan(
                newt[:, 1:129], s[:], kt[:], car_prev[:, 0:1],
                OP.add, OP.mult,
            )
            add_dep_helper(scan_i.ins, car_prev_mm.ins,
                           reason="scan initial after PE carry shift")
            if (w + 1) % resc == 0 and w + 1 < t_len:
                # per-partition chunk max -> per-instance max across groups
                pmax = p_stat.tile([npart, 1], F32, tag="pmax")
                nc.vector.tensor_reduce(pmax[:], newt[:, 1:129], AX.X, OP.max)
                pmT = p_bc.tile([1, npart], F32, tag="bc")
                t1 = nc.tensor.transpose(pmT[:], pmax[:],
                                         ident[0:npart, 0:npart])
                mxrow = p_stat.tile([1, ni], F32, tag="mxrow")
                rd2 = nc.vector.tensor_reduce(
                    mxrow[:], pmT[:].rearrange("a (g i) -> a i g", i=ni),
                    AX.X, OP.max)
                add_dep_helper(rd2.ins, t1.ins, reason="reduce after PE T1")
                mxps = p_bc.tile([ni, 1], F32, tag="bc")
                t2 = nc.tensor.transpose(mxps[:], mxrow[:], ident[0:1, 0:1])
                mxcol = p_stat.tile([ni, 1], F32, tag="mxcol")
                cpm = nc.scalar.copy(mxcol[:], mxps[:])
                add_dep_helper(cpm.ins, t2.ins, reason="copy after PE T2")
                bc = p_bc.tile([npart, 1], F32, tag="bc")
                bc_mm = nc.tensor.matmul(bc[:], bcastM[:], mxcol[:],
                                         start=True, stop=True)
                rec = p_stat.tile([npart, 1], F32, tag="rec")
                rcp = nc.vector.reciprocal(rec[:], bc[:])
                add_dep_helper(rcp.ins, bc_mm.ins,
                               reason="recip after PE broadcast")
                nc.vector.tensor_scalar_mul(newt[:, 0:129], newt[:, 0:129], rec[:])
                lgr = p_stat.tile([npart, 1], F32, tag="lgr")
                nc.scalar.activation(lgr[:], rec[:], AF.Ln)
                nc.vector.tensor_sub(cacc[:], cacc[:], lgr[:])
            car = p_carry.tile([npart, 1], F32, tag="car")
            car_mm = nc.tensor.matmul(car[:], shiftM[:], newt[:, 128:129],
                                      start=True, stop=True)
            cp = nc.scalar.copy(prev[:, 0:1], car[:])
            add_dep_helper(cp.ins, car_mm.ins,
                           reason="carry copy after PE shift")
            car_prev = car
            car_prev_mm = car_mm

        last = Etiles[nsteps % 2]
        nc.sync.dma_start(outs["EOUT"].rearrange("(a b) -> a b", b=1),
                          last[(ch - 1) * ni:ch * ni, 128:129])
        nc.sync.dma_start(outs["COUT"].rearrange("(a b) -> a b", b=1),
                          cacc[(ch - 1) * ni:ch * ni, 0:1])


def _build(t_len=T, bpc=BPC, resc=RESC, num_devices=NCORES, wave=False):
    ni = NTYPE * bpc
    nc = bacc.Bacc(
        "TRN2", target_bir_lowering=False, debug=False, num_devices=num_devices,
    )
    ins = {
        name: nc.dram_tensor(name, [bpc, t_len, D], F32, kind="ExternalInput").ap()
        for name in ("TGT", "OTH", "X")
    }
    outs = {
        "EOUT": nc.dram_tensor("EOUT", [ni], F32, kind="ExternalOutput").ap(),
        "COUT": nc.dram_tensor("COUT", [ni], F32, kind="ExternalOutput").ap(),
    }
    if wave:
        kbuf = nc.dram_tensor("KBUF", [ni, t_len + 6, t_len], F32).ap()
        with tile.TileContext(nc) as tc:
            _emit_wave(tc, ins, outs, kbuf, t_len, bpc, resc)
    else:
        kbuf = nc.dram_tensor("KBUF", [ni, t_len, t_len], F32).ap()
        with tile.TileContext(nc) as tc:
            _emit(tc, ins, outs, kbuf, t_len, bpc, resc)
    nc.compile()
    return nc


_NC = None


def _get_nc():
    global _NC
    if _NC is None:
        _NC = _build()
    return _NC


def _postprocess(results, labels):
    E = np.stack([r["EOUT"] for r in results])  # [8, 32]
    C = np.stack([r["COUT"] for r in results])  # [8, 32]
    R = -(np.log(E) + C)                        # [core, type*8+b]
    R = R.reshape(NCORES, NTYPE, BPC).transpose(1, 0, 2).reshape(NTYPE, B)
    diff = (R[0] - R[1] - 0.5 * R[2] + 0.5 * R[3]).astype(np.float32)
    lab = np.asarray(labels, dtype=np.float32)
    return np.float32(np.mean((diff - lab) ** 2, dtype=np.float32))


def kernel(TGT, OTH, X, labels):
    nc = _get_nc()
    TGT = np.ascontiguousarray(np.asarray(TGT, dtype=np.float32))
    OTH = np.ascontiguousarray(np.asarray(OTH, dtype=np.float32))
    X = np.ascontiguousarray(np.asarray(X, dtype=np.float32))
    in_maps = [
        {
            "TGT": TGT[c * BPC:(c + 1) * BPC],
            "OTH": OTH[c * BPC:(c + 1) * BPC],
            "X": X[c * BPC:(c + 1) * BPC],
        }
        for c in range(NCORES)
    ]
    res = run_bass_kernel_spmd(nc, in_maps, core_ids=list(range(NCORES)))
    return _postprocess(res.results, labels)

